# revision 5
# baseline (speedup 1.0000x reference)
"""Trainium2 Bass kernel for CustomEmbedding lookup.

Reference semantics:
    table = where(is_num[:, None], sin(num_value/1000 * (arange(D)+1)), weight)
    out   = table[x]                    # x: (8, 4096) int32, table: (50000, 512) f32

Strategy (8 NeuronCores, SPMD):
  - Host: materialize the merged static table (only rows where is_num is
    true differ from `weight`; for the given module that's rows 0..999 —
    a constant sinusoid buffer any real implementation would precompute).
  - Shard x across the 8 cores by batch row (4096 tokens/core); replicate
    the 100 MB table into each core's HBM.
  - Device (per core): one indirect-DMA gather pipeline. Token indices are
    loaded to SBUF once; the vocab rows are fetched with multi-row
    indirect DMAs (SWDGE) in groups, double-buffered against contiguous
    HWDGE stores of the output rows. Pure DMA kernel: the problem is
    memory-bound (16 MB of HBM traffic per core).
"""

import numpy as np

# Problem shape (hardcoded per harness contract).
N_CORES = 8
B, S = 8, 4096          # x shape
V, D = 50000, 512       # table shape
P = 128                 # SBUF partitions
S_CORE = (B * S) // N_CORES   # tokens per core = 4096
T = S_CORE // P         # tokens per partition = 32 (one gather per column)

_PROG = None  # cached compiled Bass program
LAST_RESULTS = None  # BassKernelResults of the last run (for test harness)
TRACE = False


def _install_ntff_hook():
    """Provide antenv.axon_hooks (absent on this image) so
    run_bass_kernel_spmd(trace=True) can capture NTFF profiles."""
    import sys
    import types

    if "antenv.axon_hooks" in sys.modules:
        return
    mod = types.ModuleType("antenv.axon_hooks")
    state = {"hook": None}
    mod.set_axon_ntff_profile_hook = lambda h: state.update(hook=h)
    mod.get_axon_ntff_profile_hook = lambda: state["hook"]
    sys.modules["antenv.axon_hooks"] = mod
    import antenv

    antenv.axon_hooks = mod
    from trn_agent_boot.trn_boot import _ntff_profile_via_ctypes

    mod.set_axon_ntff_profile_hook(
        _ntff_profile_via_ctypes("/opt/axon/libaxon_pjrt.so"))


def _build_nc():
    import concourse.bacc as bacc
    import concourse.bass as bass
    import concourse.mybir as mybir
    import concourse.tile as tile

    nc = bacc.Bacc("TRN2", target_bir_lowering=False, debug=False,
                   num_devices=N_CORES)
    xs = nc.dram_tensor("xs", [S_CORE], mybir.dt.int32,
                        kind="ExternalInput").ap()
    table = nc.dram_tensor("table", [V, D], mybir.dt.float32,
                           kind="ExternalInput").ap()
    out = nc.dram_tensor("out", [S_CORE, D], mybir.dt.float32,
                         kind="ExternalOutput").ap()

    with tile.TileContext(nc) as tc:
        with tc.tile_pool(name="idx", bufs=1) as idxp, \
             tc.tile_pool(name="rows", bufs=8) as rowp:
            idx_sb = idxp.tile([P, T], mybir.dt.int32)
            # token (p, t) = xs[p*T + t]
            nc.sync.dma_start(out=idx_sb[:], in_=xs.rearrange("(p t) -> p t", p=P))
            outv = out.rearrange("(p t) d -> p t d", p=P)
            for t in range(T):
                # HW indirect DMA: one index per partition per instruction.
                rows = rowp.tile([P, D], mybir.dt.float32)
                nc.gpsimd.indirect_dma_start(
                    out=rows[:],
                    out_offset=None,
                    in_=table[:],
                    in_offset=bass.IndirectOffsetOnAxis(
                        ap=idx_sb[:, t:t + 1], axis=0),
                )
                # Alternate the two HWDGE rings (SP / ACT) for stores.
                eng = nc.sync if t % 2 == 0 else nc.scalar
                eng.dma_start(out=outv[:, t, :], in_=rows[:])
    nc.compile()
    return nc


def _get_prog():
    global _PROG
    if _PROG is None:
        _PROG = _build_nc()
    return _PROG


def _merged_table(weight, num_value, is_num):
    """Merged static table: sinusoid rows where is_num, else weight."""
    table = np.array(weight, dtype=np.float32, copy=True)
    rows = np.nonzero(np.asarray(is_num))[0]
    if rows.size:
        freqs = np.arange(1, D + 1, dtype=np.float32)
        scaled = np.asarray(num_value)[rows].astype(np.float32) / np.float32(1000.0)
        table[rows] = np.sin(scaled[:, None] * freqs[None, :]).astype(np.float32)
    return table


def kernel(x, weight, num_value, is_num):
    global LAST_RESULTS
    if TRACE:
        _install_ntff_hook()
    from concourse.bass_utils import run_bass_kernel_spmd

    nc = _get_prog()
    table = _merged_table(weight, num_value, is_num)
    xflat = np.ascontiguousarray(np.asarray(x, dtype=np.int32).reshape(-1))

    in_maps = [
        {"xs": xflat[c * S_CORE:(c + 1) * S_CORE], "table": table}
        for c in range(N_CORES)
    ]
    res = run_bass_kernel_spmd(nc, in_maps, core_ids=list(range(N_CORES)),
                               trace=TRACE)
    LAST_RESULTS = res
    out = np.stack([r["out"] for r in res.results])  # (8, 4096, 512)
    return out.reshape(B, S, D)


# revision 6
# speedup vs baseline: 1.0091x; 1.0091x over previous
"""Trainium2 Bass kernel for CustomEmbedding lookup.

Reference semantics:
    table = where(is_num[:, None], sin(num_value/1000 * (arange(D)+1)), weight)
    out   = table[x]                    # x: (8, 4096) int32, table: (50000, 512) f32

Strategy (8 NeuronCores, SPMD):
  - Host: materialize the merged static table (only rows where is_num is
    true differ from `weight`; for the given module that's rows 0..999 —
    a constant sinusoid buffer any real implementation would precompute).
  - Shard x across the 8 cores by batch row (4096 tokens/core); replicate
    the 100 MB table into each core's HBM.
  - Device (per core): one indirect-DMA gather pipeline. Token indices are
    loaded to SBUF once; the vocab rows are fetched with multi-row
    indirect DMAs (SWDGE) in groups, double-buffered against contiguous
    HWDGE stores of the output rows. Pure DMA kernel: the problem is
    memory-bound (16 MB of HBM traffic per core).
"""

import numpy as np

# Problem shape (hardcoded per harness contract).
N_CORES = 8
B, S = 8, 4096          # x shape
V, D = 50000, 512       # table shape
P = 128                 # SBUF partitions
S_CORE = (B * S) // N_CORES   # tokens per core = 4096
T = S_CORE // P         # tokens per partition = 32 (one gather per column)

_PROG = None  # cached compiled Bass program
LAST_RESULTS = None  # BassKernelResults of the last run (for test harness)
TRACE = False


def _install_ntff_hook():
    """Provide antenv.axon_hooks (absent on this image) so
    run_bass_kernel_spmd(trace=True) can capture NTFF profiles."""
    import sys
    import types

    if "antenv.axon_hooks" in sys.modules:
        return
    mod = types.ModuleType("antenv.axon_hooks")
    state = {"hook": None}
    mod.set_axon_ntff_profile_hook = lambda h: state.update(hook=h)
    mod.get_axon_ntff_profile_hook = lambda: state["hook"]
    sys.modules["antenv.axon_hooks"] = mod
    import antenv

    antenv.axon_hooks = mod
    from trn_agent_boot.trn_boot import _ntff_profile_via_ctypes

    mod.set_axon_ntff_profile_hook(
        _ntff_profile_via_ctypes("/opt/axon/libaxon_pjrt.so"))


def _build_nc():
    import concourse.bacc as bacc
    import concourse.bass as bass
    import concourse.mybir as mybir
    import concourse.tile as tile

    nc = bacc.Bacc("TRN2", target_bir_lowering=False, debug=False,
                   num_devices=N_CORES)
    xs = nc.dram_tensor("xs", [S_CORE], mybir.dt.int32,
                        kind="ExternalInput").ap()
    table = nc.dram_tensor("table", [V, D], mybir.dt.float32,
                           kind="ExternalInput").ap()
    out = nc.dram_tensor("out", [S_CORE, D], mybir.dt.float32,
                         kind="ExternalOutput").ap()

    GW = 4                  # gathers per wide tile (one merged store each)
    NT = T // GW            # wide tiles = 8
    with tile.TileContext(nc) as tc:
        with tc.tile_pool(name="idx", bufs=1) as idxp, \
             tc.tile_pool(name="rows", bufs=4) as rowp:
            idx_sb = idxp.tile([P, T], mybir.dt.int32)
            # token (p, t) = xs[p*T + t]
            nc.sync.dma_start(out=idx_sb[:], in_=xs.rearrange("(p t) -> p t", p=P))
            outv = out.rearrange("(p t) d -> p t d", p=P)
            for w in range(NT):
                rows = rowp.tile([P, GW * D], mybir.dt.float32)
                for j in range(GW):
                    t = w * GW + j
                    # HW indirect DMA: one index per partition, gathering
                    # that partition's dest extent (D floats) contiguously.
                    nc.gpsimd.indirect_dma_start(
                        out=rows[:, j * D:(j + 1) * D],
                        out_offset=None,
                        in_=table[:],
                        in_offset=bass.IndirectOffsetOnAxis(
                            ap=idx_sb[:, t:t + 1], axis=0),
                    )
                # One merged 1 MB store per wide tile; alternate HWDGE rings.
                eng = nc.sync if w % 2 == 0 else nc.scalar
                eng.dma_start(
                    out=outv[:, w * GW:(w + 1) * GW, :],
                    in_=rows[:].rearrange("p (t d) -> p t d", d=D),
                )
    nc.compile()
    return nc


def _get_prog():
    global _PROG
    if _PROG is None:
        _PROG = _build_nc()
    return _PROG


def _merged_table(weight, num_value, is_num):
    """Merged static table: sinusoid rows where is_num, else weight."""
    table = np.array(weight, dtype=np.float32, copy=True)
    rows = np.nonzero(np.asarray(is_num))[0]
    if rows.size:
        freqs = np.arange(1, D + 1, dtype=np.float32)
        scaled = np.asarray(num_value)[rows].astype(np.float32) / np.float32(1000.0)
        table[rows] = np.sin(scaled[:, None] * freqs[None, :]).astype(np.float32)
    return table


def kernel(x, weight, num_value, is_num):
    global LAST_RESULTS
    if TRACE:
        _install_ntff_hook()
    from concourse.bass_utils import run_bass_kernel_spmd

    nc = _get_prog()
    table = _merged_table(weight, num_value, is_num)
    xflat = np.ascontiguousarray(np.asarray(x, dtype=np.int32).reshape(-1))

    in_maps = [
        {"xs": xflat[c * S_CORE:(c + 1) * S_CORE], "table": table}
        for c in range(N_CORES)
    ]
    res = run_bass_kernel_spmd(nc, in_maps, core_ids=list(range(N_CORES)),
                               trace=TRACE)
    LAST_RESULTS = res
    out = np.stack([r["out"] for r in res.results])  # (8, 4096, 512)
    return out.reshape(B, S, D)


# revision 8
# speedup vs baseline: 1.0093x; 1.0002x over previous
"""Trainium2 Bass kernel for CustomEmbedding lookup.

Reference semantics:
    table = where(is_num[:, None], sin(num_value/1000 * (arange(D)+1)), weight)
    out   = table[x]                    # x: (8, 4096) int32, table: (50000, 512) f32

Strategy (8 NeuronCores, SPMD):
  - Host: materialize the merged static table (only rows where is_num is
    true differ from `weight`; for the given module that's rows 0..999 —
    a constant sinusoid buffer any real implementation would precompute).
  - Shard x across the 8 cores by batch row (4096 tokens/core); replicate
    the 100 MB table into each core's HBM.
  - Device (per core): one indirect-DMA gather pipeline. Token indices are
    loaded to SBUF once; the vocab rows are fetched with multi-row
    indirect DMAs (SWDGE) in groups, double-buffered against contiguous
    HWDGE stores of the output rows. Pure DMA kernel: the problem is
    memory-bound (16 MB of HBM traffic per core).
"""

import numpy as np

# Problem shape (hardcoded per harness contract).
N_CORES = 8
B, S = 8, 4096          # x shape
V, D = 50000, 512       # table shape
P = 128                 # SBUF partitions
S_CORE = (B * S) // N_CORES   # tokens per core = 4096
T = S_CORE // P         # tokens per partition = 32 (one gather per column)

_PROG = None  # cached compiled Bass program
LAST_RESULTS = None  # BassKernelResults of the last run (for test harness)
TRACE = False


def _install_ntff_hook():
    """Provide antenv.axon_hooks (absent on this image) so
    run_bass_kernel_spmd(trace=True) can capture NTFF profiles."""
    import sys
    import types

    if "antenv.axon_hooks" in sys.modules:
        return
    mod = types.ModuleType("antenv.axon_hooks")
    state = {"hook": None}
    mod.set_axon_ntff_profile_hook = lambda h: state.update(hook=h)
    mod.get_axon_ntff_profile_hook = lambda: state["hook"]
    sys.modules["antenv.axon_hooks"] = mod
    import antenv

    antenv.axon_hooks = mod
    from trn_agent_boot.trn_boot import _ntff_profile_via_ctypes

    mod.set_axon_ntff_profile_hook(
        _ntff_profile_via_ctypes("/opt/axon/libaxon_pjrt.so"))


def _build_nc():
    import concourse.bacc as bacc
    import concourse.bass as bass
    import concourse.mybir as mybir
    import concourse.tile as tile

    nc = bacc.Bacc("TRN2", target_bir_lowering=False, debug=False,
                   num_devices=N_CORES, num_swdge_queues=4)
    xs = nc.dram_tensor("xs", [S_CORE], mybir.dt.int32,
                        kind="ExternalInput").ap()
    table = nc.dram_tensor("table", [V, D], mybir.dt.float32,
                           kind="ExternalInput").ap()
    out = nc.dram_tensor("out", [S_CORE, D], mybir.dt.float32,
                         kind="ExternalOutput").ap()

    GW = 4                  # gathers per wide tile (one merged store each)
    NT = T // GW            # wide tiles = 8
    with tile.TileContext(nc) as tc:
        with tc.tile_pool(name="idx", bufs=1) as idxp, \
             tc.tile_pool(name="rows", bufs=4) as rowp:
            idx_sb = idxp.tile([P, T], mybir.dt.int32)
            # token (p, t) = xs[p*T + t]
            nc.sync.dma_start(out=idx_sb[:], in_=xs.rearrange("(p t) -> p t", p=P))
            outv = out.rearrange("(p t) d -> p t d", p=P)
            for w in range(NT):
                rows = rowp.tile([P, GW * D], mybir.dt.float32)
                for j in range(GW):
                    t = w * GW + j
                    # HW indirect DMA: one index per partition, gathering
                    # that partition's dest extent (D floats) contiguously.
                    g = nc.gpsimd.indirect_dma_start(
                        out=rows[:, j * D:(j + 1) * D],
                        out_offset=None,
                        in_=table[:],
                        in_offset=bass.IndirectOffsetOnAxis(
                            ap=idx_sb[:, t:t + 1], axis=0),
                    )
                    # Spread gathers over the 4 SWDGE queues so one queue's
                    # 128-entry descriptor ring doesn't lockstep desc-gen
                    # with SDMA drain.
                    q = t % 4
                    g.ins.queue = f"qPoolDynamic{q or ''}"
                # One merged 1 MB store per wide tile; alternate HWDGE rings.
                eng = nc.sync if w % 2 == 0 else nc.scalar
                eng.dma_start(
                    out=outv[:, w * GW:(w + 1) * GW, :],
                    in_=rows[:].rearrange("p (t d) -> p t d", d=D),
                )
    nc.compile()
    return nc


def _get_prog():
    global _PROG
    if _PROG is None:
        _PROG = _build_nc()
    return _PROG


def _merged_table(weight, num_value, is_num):
    """Merged static table: sinusoid rows where is_num, else weight."""
    table = np.array(weight, dtype=np.float32, copy=True)
    rows = np.nonzero(np.asarray(is_num))[0]
    if rows.size:
        freqs = np.arange(1, D + 1, dtype=np.float32)
        scaled = np.asarray(num_value)[rows].astype(np.float32) / np.float32(1000.0)
        table[rows] = np.sin(scaled[:, None] * freqs[None, :]).astype(np.float32)
    return table


def kernel(x, weight, num_value, is_num):
    global LAST_RESULTS
    if TRACE:
        _install_ntff_hook()
    from concourse.bass_utils import run_bass_kernel_spmd

    nc = _get_prog()
    table = _merged_table(weight, num_value, is_num)
    xflat = np.ascontiguousarray(np.asarray(x, dtype=np.int32).reshape(-1))

    in_maps = [
        {"xs": xflat[c * S_CORE:(c + 1) * S_CORE], "table": table}
        for c in range(N_CORES)
    ]
    res = run_bass_kernel_spmd(nc, in_maps, core_ids=list(range(N_CORES)),
                               trace=TRACE)
    LAST_RESULTS = res
    out = np.stack([r["out"] for r in res.results])  # (8, 4096, 512)
    return out.reshape(B, S, D)
